# revision 11
# baseline (speedup 1.0000x reference)
"""Trainium2 Bass kernel for nn_CSPLayer (GNN message passing) — v2.

8-core SPMD; host sorts edges by src and shards nodes into 8 contiguous
6272-node ranges (49 aligned 128-node windows per core), so scatter-mean
needs no cross-core reduce.

Edge-layer input decomposes per edge e:
  z1[e] = P1[src[e]] + P2[dst[e]] + [lat,fd,1] @ W1cd
with P1 = NF@W1a, P2 = NF@W1b node-space tables (computed on device in a
prologue, stored bf16 in DRAM).

Per window (T 128-edge tiles, processed in 512-edge chunks):
 - ONE batched indirect-DMA gather of P2[dst] rows per window
   (amortizes the ~1us SWDGE fixed cost ~18x vs per-tile gathers).
 - z1^T [f, e] assembled in PSUM: lat term as a K=10 matmul with N=512
   moving operand; P1[src] expanded by a single one-hot matmul per
   512-edge chunk (src is window-sorted, so the one-hot [n, e] selects
   rows of the SBUF-resident node-major P1 window); gathered P2 rows
   added via per-tile transpose-accumulate matmuls (lhsT=rows,
   rhs=identity) — no DVE in the z1 assembly.
 - silu1 on [f, 512] chunks (Activation engine, PSUM->SBUF bf16).
 - z2 = ea @ W2 + b2 edge-major: lhsT = eaT chunk slices directly
   (feature-major ea needs NO transpose); bias via a K=1 ones-row
   matmul broadcasting b2 across partitions.
 - silu2 -> efs bf16; scatter-mean via one-hot matmuls where the
   one-hot is built per tile by a single fused DVE op:
   ohs[e,n] = (iota[n]==src_local[e]) * invcnt[e]  (tensor_scalar
   is_equal+mult), so agg needs no separate count/divide pass.
All matmul operands bf16 (1 cycle/row vs 4 for fp32); PSUM accumulation
stays fp32.

Node MLP feature-major on [f, 512] groups, residual add in bf16, PE
transpose + fp32 copy for the node-major output store.
"""

import numpy as np
import ml_dtypes

import concourse.bass as bass
import concourse.mybir as mybir
import concourse.tile as tile
from concourse import bacc
from concourse.bass_utils import run_bass_kernel_spmd

N_CORES = 8
H = 128
P = 128
WPC = 49            # windows per core (49*128 = 6272 nodes per core)
RPC = WPC * P       # nodes per core
NPAD = N_CORES * RPC
F32 = mybir.dt.float32
BF16 = mybir.dt.bfloat16
I32 = mybir.dt.int32
BF = ml_dtypes.bfloat16


def _chunks(T):
    out = []
    t0 = 0
    while t0 < T:
        nt = min(4, T - t0)
        out.append((t0, nt))
        t0 += nt
    return out


def _build_program(T, has_b2=True):
    nc = bacc.Bacc()
    NT = WPC * T               # edge tiles per core
    EPC = NT * P               # padded edges per core

    # ---- DRAM inputs ----
    nfTb = nc.dram_tensor("nfTb", [P, NPAD], BF16, kind="ExternalInput")
    nflb_d = nc.dram_tensor("nflb", [P, RPC], BF16, kind="ExternalInput")
    w1ab = nc.dram_tensor("w1ab", [P, H], BF16, kind="ExternalInput")
    w1bb = nc.dram_tensor("w1bb", [P, H], BF16, kind="ExternalInput")
    w1cdb = nc.dram_tensor("w1cdb", [10, H], BF16, kind="ExternalInput")
    w2bd = nc.dram_tensor("w2bd", [H, H], BF16, kind="ExternalInput")
    b2row_d = nc.dram_tensor("b2row", [1, 4 * P], BF16, kind="ExternalInput")
    nw1b = nc.dram_tensor("nw1b", [2 * H, H], BF16, kind="ExternalInput")
    nw2b = nc.dram_tensor("nw2b", [H, H], BF16, kind="ExternalInput")
    nb1c = nc.dram_tensor("nb1c", [H, 1], F32, kind="ExternalInput")
    nb2c = nc.dram_tensor("nb2c", [H, 1], F32, kind="ExternalInput")
    identb_d = nc.dram_tensor("identb", [P, P], BF16, kind="ExternalInput")
    iotab_d = nc.dram_tensor("iotab", [P, P], BF16, kind="ExternalInput")
    srccol = nc.dram_tensor("srccol", [P, NT], F32, kind="ExternalInput")   # window-local src or -1
    invc = nc.dram_tensor("invc", [P, NT], F32, kind="ExternalInput")       # 1/max(cnt,1), 0 pad
    dsti = nc.dram_tensor("dsti", [P, NT], I32, kind="ExternalInput")       # global dst idx, 0 pad
    srcrow = nc.dram_tensor("srcrow", [1, EPC], BF16, kind="ExternalInput")  # window-local src, -1 pad
    iotap_d = nc.dram_tensor("iotap", [P, 8 * P], BF16, kind="ExternalInput")  # [p, j] = p
    lat10 = nc.dram_tensor("lat10", [10, EPC], BF16, kind="ExternalInput")  # [lat6; fd3; 1], 0 pad

    p2d = nc.dram_tensor("p2d", [NPAD, H], BF16)
    out = nc.dram_tensor("out", [RPC, H], F32, kind="ExternalOutput")

    CHUNKS = _chunks(T)

    with tile.TileContext(nc) as tc:
        with (
            tc.tile_pool(name="const", bufs=1) as cpool,
            tc.tile_pool(name="persist", bufs=1) as ppool,
            tc.tile_pool(name="work", bufs=4) as wpool,
            tc.tile_pool(name="g2", bufs=3) as g2pool,
            tc.tile_pool(name="sb", bufs=3) as sbpool,
            tc.tile_pool(name="lat", bufs=3) as lpool,
            tc.tile_pool(name="edge", bufs=4) as epool,
            tc.tile_pool(name="oh", bufs=8) as opool,
            tc.tile_pool(name="mlp", bufs=3) as mpool,
            tc.tile_pool(name="psA", bufs=2, space="PSUM") as psA,
            tc.tile_pool(name="psB", bufs=2, space="PSUM") as psB,
            tc.tile_pool(name="psG", bufs=2, space="PSUM") as psG,
        ):
            # ---- constants ----
            w1a_s = cpool.tile([P, H], BF16, tag="w1a")
            nc.sync.dma_start(out=w1a_s[:], in_=w1ab[:])
            w1b_s = cpool.tile([P, H], BF16, tag="w1b")
            nc.sync.dma_start(out=w1b_s[:], in_=w1bb[:])
            w1cd_s = cpool.tile([10, H], BF16, tag="w1cd")
            nc.sync.dma_start(out=w1cd_s[:], in_=w1cdb[:])
            w2_s = cpool.tile([H, H], BF16, tag="w2")
            nc.sync.dma_start(out=w2_s[:], in_=w2bd[:])
            b2_s = cpool.tile([1, 4 * P], BF16, tag="b2")
            nc.sync.dma_start(out=b2_s[:], in_=b2row_d[:])
            nw1_s = cpool.tile([H, 2 * H], BF16, tag="nw1")
            nc.sync.dma_start(out=nw1_s[:, 0:H], in_=nw1b[0:H])
            nc.sync.dma_start(out=nw1_s[:, H:2 * H], in_=nw1b[H:2 * H])
            nw2_s = cpool.tile([H, H], BF16, tag="nw2")
            nc.sync.dma_start(out=nw2_s[:], in_=nw2b[:])
            nb1_s = cpool.tile([H, 1], F32, tag="nb1")
            nc.sync.dma_start(out=nb1_s[:], in_=nb1c[:])
            nb2_s = cpool.tile([H, 1], F32, tag="nb2")
            nc.sync.dma_start(out=nb2_s[:], in_=nb2c[:])
            id_s = cpool.tile([P, P], BF16, tag="identb")
            nc.sync.dma_start(out=id_s[:], in_=identb_d[:])
            iota_s = cpool.tile([P, P], BF16, tag="iotab")
            nc.sync.dma_start(out=iota_s[:], in_=iotab_d[:])
            src_s = cpool.tile([P, NT], F32, tag="srccol")
            nc.sync.dma_start(out=src_s[:], in_=srccol[:])
            inv_s = cpool.tile([P, NT], F32, tag="invc")
            nc.sync.dma_start(out=inv_s[:], in_=invc[:])
            dst_s = cpool.tile([P, NT], I32, tag="dsti")
            nc.sync.dma_start(out=dst_s[:], in_=dsti[:])
            iotap_s = cpool.tile([P, 8 * P], BF16, tag="iotap")
            nc.sync.dma_start(out=iotap_s[:], in_=iotap_d[:])
            ones_s = cpool.tile([1, P], BF16, tag="ones")
            nc.vector.memset(ones_s[:], 1.0)

            # ---- persistent ----
            nfl = ppool.tile([P, RPC], BF16, tag="nfl")     # local NF^T
            nc.sync.dma_start(out=nfl[:], in_=nflb_d[:])
            aggT = ppool.tile([P, RPC], BF16, tag="aggT")   # agg (mean), feature-major
            p1 = ppool.tile([P, RPC], BF16, tag="p1")       # P1 windows, node-major [n, w*H+f]

            # ---- prologue: P2 = NF @ W1b, P1 = NF_loc @ W1a (bf16 DRAM tables) ----
            GB = 8
            for g in range(NPAD // (GB * P)):
                nfb = wpool.tile([P, GB * P], BF16, tag="nfb")
                nc.sync.dma_start(out=nfb[:], in_=nfTb[:, g * GB * P:(g + 1) * GB * P])
                pt = wpool.tile([P, GB * P], BF16, tag="pt")
                ps = psA.tile([P, 8 * P], F32, tag="psA")
                for c in range(8):
                    nc.tensor.matmul(ps[:, c * P:(c + 1) * P], lhsT=nfb[:, c * P:(c + 1) * P],
                                     rhs=w1b_s[:], start=True, stop=True)
                nc.vector.tensor_copy(out=pt[:], in_=ps[:])
                nc.gpsimd.dma_start(out=p2d.ap().rearrange("(b n) f -> n b f", n=P)[:, g * GB:(g + 1) * GB, :],
                                    in_=pt[:])
            GB1 = 4
            for g in range((WPC + GB1 - 1) // GB1):
                w0 = g * GB1
                nw = min(GB1, WPC - w0)
                L = nw * P
                ps = psA.tile([P, 8 * P], F32, tag="psA")
                for j in range(nw):
                    w = w0 + j
                    nc.tensor.matmul(ps[:, j * P:(j + 1) * P], lhsT=nfl[:, w * P:(w + 1) * P],
                                     rhs=w1a_s[:], start=True, stop=True)
                nc.vector.tensor_copy(out=p1[:, w0 * P:w0 * P + L], in_=ps[:, :L])

            # ---- edge phase ----
            for w in range(WPC):
                srcb = sbpool.tile([P, T * P], BF16, tag="srcb")
                nc.sync.dma_start(out=srcb[:],
                                  in_=srcrow[0:1, w * T * P:(w + 1) * T * P].to_broadcast([P, T * P]))
                st2 = g2pool.tile([P, T * H], BF16, tag="st2")
                nc.gpsimd.indirect_dma_start(
                    out=st2[:], out_offset=None, in_=p2d[:],
                    in_offset=bass.IndirectOffsetOnAxis(ap=dst_s[:, w * T:(w + 1) * T], axis=0))
                lt = lpool.tile([10, T * P], BF16, tag="lat")
                nc.sync.dma_start(out=lt[:], in_=lat10[:, w * T * P:(w + 1) * T * P])
                aggp = psG.tile([P, 4 * P], F32, tag="aggp")

                def z2_scatter(t0, nt, eaT, off=0):
                    cw = nt * P
                    z2p = psB.tile([P, 4 * P], F32, tag="psB")
                    if has_b2:
                        nc.tensor.matmul(z2p[:, :cw], lhsT=ones_s[:], rhs=b2_s[:, :cw],
                                         start=True, stop=False)
                    for j in range(nt):
                        sl = slice(j * P, (j + 1) * P)
                        esl = slice((off + j) * P, (off + j + 1) * P)
                        nc.tensor.matmul(z2p[:, sl], lhsT=eaT[:, esl], rhs=w2_s[:],
                                         start=not has_b2, stop=(not has_b2) or (j == nt - 1))
                    efs = epool.tile([P, 4 * P], BF16, tag="efs")
                    nc.scalar.activation(efs[:, :cw], z2p[:, :cw],
                                         mybir.ActivationFunctionType.Silu)
                    for j in range(nt):
                        t = t0 + j
                        g = w * T + t
                        ohs = opool.tile([P, P], BF16, tag="ohs")
                        nc.vector.tensor_scalar(
                            out=ohs[:], in0=iota_s[:], scalar1=src_s[:, g:g + 1],
                            scalar2=inv_s[:, g:g + 1],
                            op0=mybir.AluOpType.is_equal, op1=mybir.AluOpType.mult)
                        nc.tensor.matmul(aggp[:, 0:P], lhsT=efs[:, j * P:(j + 1) * P],
                                         rhs=ohs[:], start=(t == 0), stop=(t == T - 1))

                prev = []
                t0 = 0
                while t0 < T:
                    snt = min(8, T - t0)
                    cw = snt * P
                    ohne = sbpool.tile([P, 8 * P], BF16, tag="ohne")
                    nc.vector.tensor_tensor(out=ohne[:, :cw],
                                            in0=srcb[:, t0 * P:t0 * P + cw],
                                            in1=iotap_s[:, :cw], op=mybir.AluOpType.is_equal)
                    z1p = psA.tile([P, 8 * P], F32, tag="psA")
                    for h0 in range(0, snt, 4):
                        hn = min(4, snt - h0)
                        hsl = slice(h0 * P, (h0 + hn) * P)
                        nc.tensor.matmul(z1p[:, hsl], lhsT=w1cd_s[:],
                                         rhs=lt[:, (t0 + h0) * P:(t0 + h0 + hn) * P],
                                         start=True, stop=False)
                        nc.tensor.matmul(z1p[:, hsl], lhsT=p1[:, w * P:(w + 1) * P],
                                         rhs=ohne[:, hsl], start=False, stop=False)
                        for j in range(hn):
                            t = t0 + h0 + j
                            sl = slice((h0 + j) * P, (h0 + j + 1) * P)
                            nc.tensor.matmul(z1p[:, sl], lhsT=st2[:, t * H:(t + 1) * H],
                                             rhs=id_s[:], start=False, stop=(j == hn - 1))
                    eaT = epool.tile([P, 8 * P], BF16, tag="eaT")
                    nc.scalar.activation(eaT[:, :cw], z1p[:, :cw],
                                         mybir.ActivationFunctionType.Silu)
                    for p_args in prev:
                        z2_scatter(*p_args)
                    prev = [(t0 + h0, min(4, snt - h0), eaT, h0)
                            for h0 in range(0, snt, 4)]
                    t0 += snt
                for p_args in prev:
                    z2_scatter(*p_args)
                nc.vector.tensor_copy(out=aggT[:, w * P:(w + 1) * P], in_=aggp[:, 0:P])

            # ---- node MLP (feature-major) + residual + transpose out ----
            NG = 4
            for g in range((WPC + NG - 1) // NG):
                w0 = g * NG
                nw = min(NG, WPC - w0)
                L = nw * P
                sl = slice(w0 * P, w0 * P + L)
                h1p = psA.tile([P, 8 * P], F32, tag="psA")
                nc.tensor.matmul(h1p[:, :L], lhsT=nw1_s[:, 0:H], rhs=nfl[:, sl],
                                 start=True, stop=False)
                nc.tensor.matmul(h1p[:, :L], lhsT=nw1_s[:, H:2 * H], rhs=aggT[:, sl],
                                 start=False, stop=True)
                h1 = mpool.tile([P, NG * P], BF16, tag="h1")
                nc.scalar.activation(h1[:, :L], h1p[:, :L],
                                     mybir.ActivationFunctionType.Silu, bias=nb1_s[:])
                h2p = psB.tile([P, NG * P], F32, tag="psB")
                nc.tensor.matmul(h2p[:, :L], lhsT=nw2_s[:], rhs=h1[:, :L],
                                 start=True, stop=True)
                h2 = mpool.tile([P, NG * P], BF16, tag="h2")
                nc.scalar.activation(h2[:, :L], h2p[:, :L],
                                     mybir.ActivationFunctionType.Silu, bias=nb2_s[:])
                oT = mpool.tile([P, NG * P], BF16, tag="oT")
                nc.vector.tensor_tensor(out=oT[:, :L], in0=h2[:, :L], in1=nfl[:, sl],
                                        op=mybir.AluOpType.add)
                obp = psA.tile([P, 8 * P], F32, tag="psA")
                for j in range(nw):
                    js = slice(j * P, (j + 1) * P)
                    nc.tensor.matmul(obp[:, js], lhsT=oT[:, js], rhs=id_s[:],
                                     start=True, stop=True)
                ob = mpool.tile([P, NG * P], F32, tag="ob")
                nc.vector.tensor_copy(out=ob[:, :L], in_=obp[:, :L])
                nc.sync.dma_start(
                    out=out.ap().rearrange("(b n) f -> n b f", n=P)[:, w0:w0 + nw, :],
                    in_=ob[:, :L])

    nc.compile()
    return nc


def _prep_core(k, src, dst, lat10_all, invc_e, T):
    """Build core k's padded per-edge arrays from globally sorted edge data."""
    r0, r1 = k * RPC, (k + 1) * RPC
    e0, e1 = np.searchsorted(src, [r0, r1])
    s, d = src[e0:e1], dst[e0:e1]
    l10 = lat10_all[:, e0:e1]
    ic = invc_e[e0:e1]
    EPC = WPC * T * P
    srcloc = np.full(EPC, -1.0, np.float32)
    dsti = np.zeros(EPC, np.int32)
    invc = np.zeros(EPC, np.float32)
    l10p = np.zeros((10, EPC), np.float32)
    wid = (s - r0) // P
    bounds = np.searchsorted(wid, np.arange(WPC + 1))
    for w in range(WPC):
        a, b = bounds[w], bounds[w + 1]
        n = b - a
        if n > T * P:
            raise RuntimeError(f"window overflow: {n} > {T * P}")
        o = w * T * P
        srcloc[o:o + n] = (s[a:b] - r0 - w * P).astype(np.float32)
        dsti[o:o + n] = d[a:b]
        invc[o:o + n] = ic[a:b]
        l10p[:, o:o + n] = l10[:, a:b]
    nt = WPC * T
    return (srcloc.reshape(nt, P).T.copy(),
            srcloc[None, :].astype(BF),
            dsti.reshape(nt, P).T.copy(),
            invc.reshape(nt, P).T.copy(),
            l10p.astype(BF))


def kernel(**inputs):
    inp = {k: np.asarray(v) for k, v in inputs.items()}
    nf = inp["node_features"].astype(np.float32)
    lattices = inp["lattices"].astype(np.float32)
    fd = inp["frac_diff"].astype(np.float32)
    ei = inp["edge_index"].astype(np.int64)
    e2g = inp["edge2graph"].astype(np.int64)
    e_w1, e_b1 = inp["e_w1"].astype(np.float32), inp["e_b1"].astype(np.float32)
    e_w2, e_b2 = inp["e_w2"].astype(np.float32), inp["e_b2"].astype(np.float32)
    n_w1, n_b1 = inp["n_w1"].astype(np.float32), inp["n_b1"].astype(np.float32)
    n_w2, n_b2 = inp["n_w2"].astype(np.float32), inp["n_b2"].astype(np.float32)

    N, Hf = nf.shape
    E = ei.shape[1]
    assert Hf == H and N <= NPAD

    perm = np.argsort(ei[0], kind="stable")
    src = ei[0][perm].astype(np.int64)
    dst = ei[1][perm].astype(np.int32)
    e2gs = e2g[perm]
    fds = fd[perm]
    lat10_all = np.concatenate(
        [lattices[e2gs].T.astype(np.float32),
         fds.T.astype(np.float32),
         np.ones((1, E), np.float32)], axis=0)            # [10, E]
    cnt = np.bincount(src, minlength=N).astype(np.float32)
    invc_e = (1.0 / np.maximum(cnt, 1.0))[src].astype(np.float32)

    r_all = src // P
    wcnt = np.bincount(r_all, minlength=N_CORES * WPC)
    T = max(18, int(np.ceil(wcnt.max() / P)))

    has_b2 = bool(np.any(e_b2))
    nc = _build_program(T, has_b2=has_b2)

    nfT = np.zeros((H, NPAD), np.float32)
    nfT[:, :N] = nf.T
    nfTb = nfT.astype(BF)
    w1cd = np.concatenate([e_w1[2 * H:], e_b1[None, :]], axis=0)

    common = dict(
        nfTb=nfTb,
        w1ab=e_w1[0:H].astype(BF), w1bb=e_w1[H:2 * H].astype(BF),
        w1cdb=w1cd.astype(BF), w2bd=e_w2.astype(BF),
        b2row=np.tile(e_b2, 4)[None, :].astype(BF),
        nw1b=n_w1.astype(BF), nw2b=n_w2.astype(BF),
        nb1c=n_b1[:, None].astype(np.float32), nb2c=n_b2[:, None].astype(np.float32),
        identb=np.eye(P, dtype=np.float32).astype(BF),
        iotab=np.tile(np.arange(P, dtype=np.float32)[None, :], (P, 1)).astype(BF),
        iotap=np.tile(np.arange(P, dtype=np.float32)[:, None], (1, 8 * P)).astype(BF),
    )
    in_maps = []
    for k in range(N_CORES):
        srccol, srow, dcol, iccol, l10p = _prep_core(k, src, dst, lat10_all, invc_e, T)
        in_maps.append(dict(
            common,
            nflb=np.ascontiguousarray(nfTb[:, k * RPC:(k + 1) * RPC]),
            srccol=srccol, srcrow=srow, dsti=dcol, invc=iccol, lat10=l10p,
        ))

    import os
    r = run_bass_kernel_spmd(nc, in_maps, core_ids=list(range(N_CORES)),
                             trace=bool(int(os.environ.get("K_TRACE", "0"))))
    out = np.concatenate([r.results[k]["out"] for k in range(N_CORES)], axis=0)[:N]
    kernel.last_exec_ns = r.exec_time_ns
    kernel.last_mean_ns = r.mean_exec_time_ns
    kernel.last_T = T
    return out.astype(np.float32)
